# revision 57
# baseline (speedup 1.0000x reference)
"""GATv2 actor layer (nn_GATv2Actor) on 8 TRN2 NeuronCores via Bass/Tile.

Self-contained: kernel(**inputs) takes the full (unsharded) inputs of
reference.setup_inputs() and returns the full [50000, 4] float32 output.

Distribution strategy (edge-parallel by destination-node range):
  - node n is owned by core n // 6250; each core handles all edges whose
    destination lies in its range (plus its self-loops), so the segment
    softmax and the scatter-add are fully core-local and the final
    output rows are disjoint (host just concatenates - no collective).

Per-core program (v3 - no node-table phase; per-edge compute on PE):
  - hgT [feat, edge] fp16 arrives directly via TRANSPOSED dma_gather of
    raw h rows (128 elems each) - no PE transpose, no PSUM evacuation.
  - one-hot edge<->dst matrices (oh for the scatter, ohT for the a_dst
    select) are pure index data: host-built, streamed as fp8 {0,1}.
  - per tile: stT[hd,e] = W_src^T@hgT + adst_j@ohT (2 matmuls, f32 PSUM)
    -> relu -> lr fp16 (engine round-robin Pool/Act/DVE); lgp[e,2] =
    lr^T@(0.8*attn); lg = lgp + csum (host linear leaky_relu term);
    exx[e,2,64] = Exp(lg) broadcast (Act); v[e,128] = hgT^T@Wv (PE);
    wt = v * exx (TT from PSUM, Pool/DVE round-robin); scatter
    ps[d,0:130] += oh^T @ [wt | ex] per dst-block (PE).
  - phase C: batched softmax denominators, out MLP, phase softmax
    (unchanged from v2).

SPMD uniformity: one program runs on all 8 cores; per-(block,stream)
tile counts are padded to the max over cores. int16 gather indices
limit tables to 32767 rows, so edges are split into two streams by
src < 32768 gathering from two base offsets of the h16 table.
"""
import sys

import numpy as np

sys.path.insert(0, "/opt/trn_rl_repo")

import ml_dtypes  # noqa: E402

import concourse.bass as bass  # noqa: E402
import concourse.tile as tile  # noqa: E402
from concourse import bacc, mybir  # noqa: E402
from concourse.bass_utils import run_bass_kernel_spmd  # noqa: E402

FP16 = mybir.dt.float16
F32 = mybir.dt.float32
FP8 = mybir.dt.float8e4
I16 = mybir.dt.int16
AT = mybir.AluOpType
ACTF = mybir.ActivationFunctionType

F = 128      # feature dim
H = 2        # heads
D = 64       # head dim
P_OUT = 4    # phases
N_CORES = 8


def _to_fp8_01(mask):
    """bool -> float8_e4m3 {0.0, 1.0} without a slow dtype conversion."""
    return (mask.astype(np.uint8) * 0x38).view(ml_dtypes.float8_e4m3)


def prep(h_int, edge_index, pair_W, pair_b, attn_w, value_W, out_W, out_b,
         phase_W, phase_b, n_cores=N_CORES, G=24, split=32768):
    """Host-side index preprocessing + input packing. Returns (meta, in_maps)."""
    h = np.asarray(h_int, np.float32)
    ei = np.asarray(edge_index)
    pair_W = np.asarray(pair_W, np.float32)
    pair_b = np.asarray(pair_b, np.float32)
    attn_w = np.asarray(attn_w, np.float32)
    value_W = np.asarray(value_W, np.float32)
    out_W = np.asarray(out_W, np.float32)
    out_b = np.asarray(out_b, np.float32)
    phase_W = np.asarray(phase_W, np.float32)
    phase_b = np.asarray(phase_b, np.float32)
    N = h.shape[0]
    assert N % n_cores == 0
    NPC = N // n_cores
    NBLK = (NPC + 127) // 128
    NPAD = ((N + 127) // 128) * 128
    assert NPAD - split < 32768 and split < 32768 + 1

    # self-loops are handled per dst-block in the aggregation (v_loc matmul
    # + host exp_self scale), not as edges
    src = ei[0].astype(np.int64)
    dst = ei[1].astype(np.int64)
    core = dst // NPC

    # per-node linear logit terms: 0.2 * attn_h . a_src / 0.2 * attn_h . (a_dst+b)
    a_src_n = np.einsum('nf,hfd->nhd', h, pair_W[:, :F, :])
    a_dst_n = np.einsum('nf,hfd->nhd', h, pair_W[:, F:, :]) + pair_b[None]
    csrc_n = 0.2 * np.einsum('nhd,hd->nh', a_src_n, attn_w)     # [N, H]
    cdst_n = 0.2 * np.einsum('nhd,hd->nh', a_dst_n, attn_w)     # [N, H]
    csum_e = (csrc_n[src] + cdst_n[dst]).astype(np.float16)     # [E, H]

    # self-loop attention weight per node: exp(attn . leaky_relu(a_src+a_dst))
    st_self = a_src_n + a_dst_n                                  # [N, H, D]
    hid_self = np.where(st_self > 0, st_self, 0.2 * st_self)
    est_n = np.exp(np.einsum('nhd,hd->nh', hid_self, attn_w))    # [N, H]

    percore = []
    counts = np.zeros((n_cores, 2, NBLK), np.int64)
    for c in range(n_cores):
        m = core == c
        es = src[m]
        ed = dst[m] - c * NPC
        cs = csum_e[m]
        o = np.lexsort((es, ed))
        es, ed, cs = es[o], ed[o], cs[o]
        lo = es < split
        percore.append((es, ed, cs, lo))
        for si in range(2):
            msk = lo if si == 0 else ~lo
            counts[c, si] = np.bincount(ed[msk] // 128, minlength=NBLK)
    T = np.ceil(counts.max(axis=0) / 128.0).astype(np.int64)  # [2, NBLK]
    tiles = T.sum(axis=1)
    L = tiles * 128
    base_tile = np.zeros((2, NBLK + 1), np.int64)
    base_tile[:, 1:] = np.cumsum(T, axis=1)

    f16 = np.float16
    # weights: columns are (head, d) flattened, matching agg.reshape(n,-1)
    W_src = np.concatenate([pair_W[0, :F], pair_W[1, :F]], axis=1).astype(f16)
    Wv = np.concatenate([value_W[0], value_W[1]], axis=1).astype(f16)
    ident16 = np.eye(128, dtype=f16)
    S16 = np.zeros((128, H), f16)
    for hh in range(H):
        S16[hh * D:(hh + 1) * D, hh] = (0.8 * attn_w[hh]).astype(f16)
    out_Wt = np.asarray(out_W, f16)
    out_b_c = out_b.reshape(128, 1).copy()
    phase_Wt = np.asarray(phase_W, f16)
    phase_b_bc = np.broadcast_to(phase_b, (128, P_OUT)).copy()

    hp = np.zeros((NPAD, F), np.float32)
    hp[:N] = h
    h16 = hp.astype(f16)
    h16_lo = np.ascontiguousarray(h16[:split])
    h16_hi = np.ascontiguousarray(h16[split:])

    # host adst table (a_dst + bias), per-core local blocks [128, NBLK, 128]
    adp = np.zeros((NPAD, H * D), np.float32)
    adp[:N] = a_dst_n.reshape(N, H * D)

    # zero-padded 130-col copies of Wv per head + 2x2 identity for the
    # self-loop matmuls (each covers the full agg range so PSUM group
    # init is a single full-width start=True write)
    Wv0x = np.zeros((128, 130), f16)
    Wv0x[:, 0:D] = Wv[:, 0:D]
    Wv1x = np.zeros((128, 130), f16)
    Wv1x[:, D:2 * D] = Wv[:, D:2 * D]
    I2x = np.zeros((2, 130), f16)
    I2x[0, 128] = 1.0
    I2x[1, 129] = 1.0

    # phase_W copies padded to a 16-col stripe per block-within-group so
    # the four phase matmuls of a group accumulate into one PSUM region
    # with a single full-width start=True (sim/hw-safe group init)
    phWx = np.zeros((4, 128, 16), f16)
    for i in range(4):
        phWx[i, :, i * P_OUT:(i + 1) * P_OUT] = phase_W.astype(f16)

    shared = dict(h16_lo=h16_lo, h16_hi=h16_hi, W_src=W_src, Wv=Wv,
                  Wv0x=Wv0x, Wv1x=Wv1x, I2x=I2x,
                  phWx=phWx.reshape(4 * 128, 16),
                  ident16=ident16, S16=S16, out_Wt=out_Wt,
                  out_b=out_b_c, phase_b_bc=phase_b_bc)

    dcols = np.arange(128, dtype=np.int64)
    in_maps = []
    for c in range(n_cores):
        es, ed, cs, lo = percore[c]
        m = {}
        for si in range(2):
            msk = lo if si == 0 else ~lo
            es_s, ed_s, cs_s = es[msk], ed[msk], cs[msk]
            gidx = np.zeros(L[si], np.int16)
            drel = np.full(L[si], -1.0, np.float32)
            csum = np.zeros((L[si], H), np.float16)
            starts = np.searchsorted(ed_s, np.arange(NBLK + 1) * 128)
            for j in range(NBLK):
                seg = slice(starts[j], starts[j + 1])
                n = starts[j + 1] - starts[j]
                b0 = base_tile[si, j] * 128
                gidx[b0:b0 + n] = (es_s[seg] - (split if si else 0)).astype(np.int16)
                drel[b0:b0 + n] = (ed_s[seg] - j * 128).astype(np.float32)
                csum[b0:b0 + n] = cs_s[seg]
            m[f"gw{si}"] = np.tile(gidx.reshape(-1, 16).T, (8, 1)).copy()
            m[f"csum{si}"] = np.ascontiguousarray(
                csum.reshape(-1, 128, H).transpose(1, 0, 2).reshape(128, -1))
            d2 = drel.reshape(-1, 128)                      # [T, e]
            oh = _to_fp8_01(d2[:, :, None] == dcols[None, None, :])  # [T,e,d]
            m[f"oh8_{si}"] = np.ascontiguousarray(
                oh.transpose(1, 0, 2)).reshape(128, -1)
            m[f"ohT8_{si}"] = np.ascontiguousarray(
                oh.transpose(2, 0, 1)).reshape(128, -1)
        adl = adp[c * NPC:c * NPC + NBLK * 128]
        m["adst"] = np.ascontiguousarray(
            adl.reshape(NBLK, 128, 128).transpose(1, 0, 2)
            .reshape(128, -1).astype(f16))
        # self-loop data: local h pre-scaled by exp_self per head
        # (feature-major), plus exp_self itself for the denominators
        hl = np.zeros((NBLK * 128, F), np.float32)
        hl[:NPC] = h[c * NPC:(c + 1) * NPC]
        estl = np.zeros((NBLK * 128, H), np.float32)
        estl[:NPC] = est_n[c * NPC:(c + 1) * NPC]
        for hh in range(H):
            m[f"hsc{hh}"] = np.ascontiguousarray(
                (hl * estl[:, hh:hh + 1]).T.astype(f16))
        m["estT2"] = np.ascontiguousarray(estl.T.astype(f16))
        m.update(shared)
        in_maps.append(m)

    meta = dict(N=N, NPC=NPC, NBLK=NBLK, NPAD=NPAD, split=split, G=G,
                T=T, tiles=tiles, L=L, base_tile=base_tile,
                n_cores=n_cores)
    return meta, in_maps


def build(meta):
    NPC, NBLK, NPAD = meta["NPC"], meta["NBLK"], meta["NPAD"]
    split, G = meta["split"], meta["G"]
    T, tiles, L = meta["T"], meta["tiles"], meta["L"]
    base_tile = meta["base_tile"]
    last_rows = NPC - (NBLK - 1) * 128

    # tile index -> destination block, per stream (compile-time constants,
    # shared across cores by construction of T)
    tile2blk = [[], []]
    for s in range(2):
        for j in range(NBLK):
            tile2blk[s].extend([j] * int(T[s][j]))

    nc = bacc.Bacc(None, target_bir_lowering=False, debug=False,
                   dynamic_dma_scratch_size=81920)

    hlo_d = nc.dram_tensor("h16_lo", [split, F], FP16, kind="ExternalInput")
    hhi_d = nc.dram_tensor("h16_hi", [NPAD - split, F], FP16,
                           kind="ExternalInput")
    gw_d = [nc.dram_tensor(f"gw{s}", [128, int(L[s]) // 16], I16,
                           kind="ExternalInput") for s in range(2)]
    oh8_d = [nc.dram_tensor(f"oh8_{s}", [128, int(tiles[s]) * 128], FP8,
                            kind="ExternalInput") for s in range(2)]
    ohT8_d = [nc.dram_tensor(f"ohT8_{s}", [128, int(tiles[s]) * 128], FP8,
                             kind="ExternalInput") for s in range(2)]
    csum_d = [nc.dram_tensor(f"csum{s}", [128, int(tiles[s]) * H], FP16,
                             kind="ExternalInput") for s in range(2)]
    adst_d = nc.dram_tensor("adst", [128, NBLK * 128], FP16,
                            kind="ExternalInput")
    hsc_d = [nc.dram_tensor(f"hsc{hh}", [128, NBLK * 128], FP16,
                            kind="ExternalInput") for hh in range(H)]
    estT2_d = nc.dram_tensor("estT2", [2, NBLK * 128], FP16,
                             kind="ExternalInput")
    Wv0x_d = nc.dram_tensor("Wv0x", [128, 130], FP16, kind="ExternalInput")
    Wv1x_d = nc.dram_tensor("Wv1x", [128, 130], FP16, kind="ExternalInput")
    I2x_d = nc.dram_tensor("I2x", [2, 130], FP16, kind="ExternalInput")
    Wsrc_d = nc.dram_tensor("W_src", [128, 128], FP16, kind="ExternalInput")
    Wv_d = nc.dram_tensor("Wv", [128, 128], FP16, kind="ExternalInput")
    S_d = nc.dram_tensor("S16", [128, H], FP16, kind="ExternalInput")
    ident_d = nc.dram_tensor("ident16", [128, 128], FP16, kind="ExternalInput")
    outW_d = nc.dram_tensor("out_Wt", [128, 128], FP16, kind="ExternalInput")
    outb_d = nc.dram_tensor("out_b", [128, 1], F32, kind="ExternalInput")
    phWx_d = nc.dram_tensor("phWx", [4 * 128, 16], FP16, kind="ExternalInput")
    phb_d = nc.dram_tensor("phase_b_bc", [128, P_OUT], F32, kind="ExternalInput")

    out_d = nc.dram_tensor("out", [NPC, P_OUT], F32, kind="ExternalOutput")

    with tile.TileContext(nc) as tc:
        with tc.tile_pool(name="consts", bufs=1) as pc:
            def cload(name, dram, shape, dtype):
                t = pc.tile(shape, dtype, tag=name)
                nc.sync.dma_start(t[:], dram[:])
                return t
            # gw first, split so the first chunks' indices land in ~100ns
            # and the opening gathers aren't gated on the full-table load
            GW_HEAD = 4 * G * 8   # first 4 chunks per stream
            gw = []
            for s in range(2):
                t = pc.tile([128, int(L[s]) // 16], I16, tag=f"gw{s}",
                            name=f"gw{s}")
                hd = min(GW_HEAD, int(L[s]) // 16)
                nc.sync.dma_start(t[:, 0:hd], gw_d[s][:, 0:hd])
                if hd < int(L[s]) // 16:
                    nc.sync.dma_start(t[:, hd:], gw_d[s][:, hd:])
                gw.append(t)
            Wsrc = cload("Wsrc", Wsrc_d, [128, 128], FP16)
            Wv = cload("Wv", Wv_d, [128, 128], FP16)
            adst = pc.tile([128, NBLK, 128], FP16, tag="adst")
            nc.sync.dma_start(
                adst[:], adst_d[:].rearrange("p (j n) -> p j n", j=NBLK))
            S = cload("S", S_d, [128, H], FP16)
            csum = [pc.tile([128, int(tiles[s]), H], FP16, tag=f"csum{s}",
                            name=f"csum{s}") for s in range(2)]
            for s in range(2):
                nc.sync.dma_start(
                    csum[s][:],
                    csum_d[s][:].rearrange("p (t h) -> p t h", h=H))
            Wv0x = cload("Wv0x", Wv0x_d, [128, 130], FP16)
            Wv1x = cload("Wv1x", Wv1x_d, [128, 130], FP16)
            I2x = cload("I2x", I2x_d, [2, 130], FP16)
            ident = cload("ident", ident_d, [128, 128], FP16)
            ident32 = pc.tile([128, 128], F32, tag="ident32")
            nc.vector.tensor_copy(ident32[:], ident[:])
            outW = cload("outW", outW_d, [128, 128], FP16)
            outb = cload("outb", outb_d, [128, 1], F32)
            phWx = pc.tile([128, 4, 16], FP16, tag="phWx")
            nc.sync.dma_start(phWx[:],
                              phWx_d[:].rearrange("(i p) f -> p i f", i=4))
            phb = cload("phb", phb_d, [128, P_OUT], F32)
            aggAll = pc.tile([128, NBLK, 130], FP16, tag="aggAll")
            phall = pc.tile([128, NBLK, P_OUT], F32, tag="phall")

            h_base = [hlo_d[:], hhi_d[:]]

            with tc.tile_pool(name="pg_hgT", bufs=4) as pg_hgT, \
                 tc.tile_pool(name="p_oh", bufs=4) as p_oh, \
                 tc.tile_pool(name="p_ohT", bufs=3) as p_ohT, \
                 tc.tile_pool(name="p_lr", bufs=3) as p_lr, \
                 tc.tile_pool(name="p_lg", bufs=3) as p_lg, \
                 tc.tile_pool(name="p_exx", bufs=3) as p_exx, \
                 tc.tile_pool(name="p_wt", bufs=2) as p_wt, \
                 tc.tile_pool(name="p_self", bufs=3) as p_self, \
                 tc.tile_pool(name="pc_work", bufs=2) as pc_work, \
                 tc.tile_pool(name="ps_stT", bufs=3, space="PSUM") as ps_stT, \
                 tc.tile_pool(name="ps_v", bufs=2, space="PSUM") as ps_v, \
                 tc.tile_pool(name="ps_lg", bufs=1, space="PSUM") as ps_lg, \
                 tc.tile_pool(name="ps_agg", bufs=2, space="PSUM") as ps_agg:

                # relu / wt engine round-robin (tunable balance knobs)
                def relu_pool(out, in_):
                    nc.gpsimd.tensor_scalar(out, in_, 0.0, None, op0=AT.max)

                def relu_act(out, in_):
                    nc.scalar.activation(out, in_, ACTF.Relu)

                def relu_dve(out, in_):
                    nc.vector.tensor_scalar(out, in_, 0.0, None, op0=AT.max)

                # Pool cannot touch PSUM (walrus birverifier) - PSUM
                # evacuations go DVE/Act only.
                relu_rr = [relu_act, relu_act, relu_act, relu_dve]

                def wt_dve(out, a, b):
                    nc.vector.tensor_tensor(out, a, b, op=AT.mult)

                def wt_pool(out, a, b):
                    nc.gpsimd.tensor_tensor(out, a, b, op=AT.mult)

                wt_rr = [wt_dve, wt_dve, wt_dve, wt_dve]

                rr_ctr = [0, 0, 0, 0]
                state = [dict(), dict()]

                def stage1(s, ci):
                    if ci in state[s]:
                        return
                    t0 = ci * G
                    g = min(G, int(tiles[s]) - t0)
                    hgT = pg_hgT.tile([128, g, 128], FP16, tag="hgT",
                                      name="hgT")
                    GSUB = 4  # transpose-mode gathers fail above 512 idxs
                    for k in range(0, g, GSUB):
                        gs = min(GSUB, g - k)
                        ne = gs * 128
                        nc.gpsimd.dma_gather(
                            hgT[:, k:k + gs, :]
                            .rearrange("p t e -> p (t e)")
                            .rearrange("p (o n) -> p o n", o=1),
                            h_base[s],
                            gw[s][:, (t0 + k) * 8:(t0 + k + gs) * 8],
                            ne, ne, 128, transpose=True)
                    oh8 = p_oh.tile([128, g, 128], FP8, tag="oh8",
                                    name="oh8")
                    nc.sync.dma_start(
                        oh8[:], oh8_d[s][:, t0 * 128:(t0 + g) * 128]
                        .rearrange("p (t e) -> p t e", t=g))
                    ohT8 = p_ohT.tile([128, g, 128], FP8, tag="ohT8",
                                      name="ohT8")
                    nc.sync.dma_start(
                        ohT8[:], ohT8_d[s][:, t0 * 128:(t0 + g) * 128]
                        .rearrange("p (t e) -> p t e", t=g))
                    state[s][ci] = dict(stg=1, t0=t0, g=g, hgT=hgT,
                                        oh=oh8, ohT=ohT8)

                def stage2(s, ci):
                    stage1(s, ci)
                    st = state[s][ci]
                    if st["stg"] >= 2:
                        return
                    t0, g, hgT, ohT8 = st["t0"], st["g"], st["hgT"], st["ohT"]
                    lr = p_lr.tile([128, g, 128], FP16, tag="lr", name="lr")
                    lgp = ps_lg.tile([128, g, H], F32, tag="lgp", name="lgp")
                    nk = (g + 3) // 4

                    def emit_lgp(kidx):
                        k = kidx * 4
                        kk = min(4, g - k)
                        for i in range(kk):
                            t = k + i
                            nc.tensor.matmul(lgp[:, t, :], lr[:, t, :], S[:],
                                             start=True, stop=True)

                    # lgp trails the relu by one 4-tile batch so the PE queue
                    # never parks on an in-flight relu
                    for kidx in range(nk):
                        k = kidx * 4
                        kk = min(4, g - k)
                        stT = ps_stT.tile([128, kk, 128], F32, tag="stT",
                                          name="stT")
                        # one W_src matmul covers the whole 4-tile batch
                        nc.tensor.matmul(
                            stT[:], Wsrc[:],
                            hgT[:, k:k + kk, :],
                            start=True, stop=False, skip_group_check=True)
                        for i in range(kk):
                            t = k + i
                            j = tile2blk[s][t0 + t]
                            nc.tensor.matmul(stT[:, i, :], adst[:, j, :],
                                             ohT8[:, t, :],
                                             start=False, stop=(i == kk - 1),
                                             skip_group_check=True)
                        relu_rr[rr_ctr[0] % len(relu_rr)](lr[:, k:k + kk, :],
                                                          stT[:])
                        rr_ctr[0] += 1
                        if kidx >= 1:
                            emit_lgp(kidx - 1)
                    emit_lgp(nk - 1)
                    # evacuate lgp immediately (adds the host linear
                    # leaky_relu term) so ps_lg recycles with one buf
                    lg = p_lg.tile([128, g, H], FP16, tag="lg", name="lg")
                    nc.vector.tensor_tensor(lg[:], lgp[:],
                                            csum[s][:, t0:t0 + g, :],
                                            op=AT.add)
                    st["lg"] = lg
                    st["stg"] = 2

                def stage3a(s, ci):
                    stage2(s, ci)
                    st = state[s][ci]
                    if st["stg"] >= 3:
                        return
                    g, lg = st["g"], st["lg"]
                    ex32 = p_lg.tile([128, g, H], FP16, tag="ex32",
                                     name="ex32")
                    nc.scalar.activation(ex32[:], lg[:], ACTF.Exp)
                    exx = p_exx.tile([128, g, H, D], FP16, tag="exx",
                                     name="exx")
                    bc = (ex32[:].rearrange("p g (h o) -> p g h o", o=1)
                          .to_broadcast((128, g, H, D)))
                    if rr_ctr[2] % 5 < 3:
                        nc.gpsimd.tensor_copy(exx[:], bc)
                    else:
                        nc.scalar.copy(exx[:], bc)
                    rr_ctr[2] += 1
                    st["exx"], st["ex32"] = exx, ex32
                    st["stg"] = 3

                def stage3b(s, ci):
                    stage3a(s, ci)
                    st = state[s][ci]
                    if st["stg"] >= 4:
                        return st["oh"], st["wt"]
                    g, hgT, exx = st["g"], st["hgT"], st["exx"]
                    wt = p_wt.tile([128, g, 130], FP16, tag="wt", name="wt")
                    nk = (g + 3) // 4
                    for kidx in range(nk):
                        k = kidx * 4
                        kk = min(4, g - k)
                        v_ps = ps_v.tile([128, kk, 128], F32, tag="v_ps",
                                         name="v_ps")
                        for i in range(kk):
                            nc.tensor.matmul(v_ps[:, i, :], hgT[:, k + i, :],
                                             Wv[:], start=True, stop=True)
                        wt_rr[rr_ctr[1] % len(wt_rr)](
                            wt[:, k:k + kk, 0:128], v_ps[:],
                            exx[:, k:k + kk, :, :]
                            .rearrange("p g h d -> p g (h d)"))
                        rr_ctr[1] += 1
                    nc.gpsimd.tensor_copy(wt[:, :, 128:130], st["ex32"][:])
                    st["wt"] = wt
                    st["stg"] = 4
                    return st["oh"], st["wt"]

                # self-loop per-block prefetch (pre-scaled hloc + exp_self)
                self_tiles = [None] * NBLK

                def self_fetch(j):
                    if j >= NBLK or self_tiles[j] is not None:
                        return
                    hs0 = p_self.tile([128, 128], FP16, tag="hs0", name="hs0")
                    nc.sync.dma_start(hs0[:],
                                      hsc_d[0][:, j * 128:(j + 1) * 128])
                    hs1 = p_self.tile([128, 128], FP16, tag="hs1", name="hs1")
                    nc.sync.dma_start(hs1[:],
                                      hsc_d[1][:, j * 128:(j + 1) * 128])
                    es = p_self.tile([2, 128], FP16, tag="es", name="es")
                    nc.sync.dma_start(es[:],
                                      estT2_d[:, j * 128:(j + 1) * 128])
                    self_tiles[j] = (hs0, hs1, es)

                # global tile sequence in scatter order, for chunk prefetch
                seq = [(j, s, int(base_tile[s, j]) + t)
                       for j in range(NBLK)
                       for s in range(2)
                       for t in range(int(T[s][j]))]
                L3A, L2, L1 = 12, 32, 40   # lookahead in tiles per stage
                si = 0
                self_fetch(0)
                self_fetch(1)
                for j in range(NBLK):
                    self_fetch(j + 2)
                    n_ev = int(T[0][j] + T[1][j]) + 3
                    ps = ps_agg.tile([128, 130], F32, tag="ps")
                    hs0, hs1, es = self_tiles[j]
                    self_tiles[j] = None
                    # self-loop contribution: agg[d] += exp_self_h[d] *
                    # (h[d] @ Wv_h), denominators += exp_self, each matmul
                    # spanning the full 130 cols via zero-padded weights
                    nc.tensor.matmul(ps[:], hs0[:], Wv0x[:],
                                     start=True, stop=False,
                                     skip_group_check=True)
                    nc.tensor.matmul(ps[:], hs1[:], Wv1x[:],
                                     start=False, stop=False,
                                     skip_group_check=True)
                    nc.tensor.matmul(ps[:], es[:], I2x[:],
                                     start=False, stop=False,
                                     skip_group_check=True)
                    ev = 3
                    for s in range(2):
                        for t in range(int(T[s][j])):
                            gt = int(base_tile[s, j]) + t
                            if si + L3A < len(seq):
                                _, s3a, gt3a = seq[si + L3A]
                                stage3a(s3a, gt3a // G)
                            if si + L2 < len(seq):
                                _, s2, gt2 = seq[si + L2]
                                stage2(s2, gt2 // G)
                            if si + L1 < len(seq):
                                _, s3, gt3 = seq[si + L1]
                                stage1(s3, gt3 // G)
                            oh, wt = stage3b(s, gt // G)
                            si += 1
                            off = gt % G
                            nc.tensor.matmul(ps[:], oh[:, off, :],
                                             wt[:, off, 0:130],
                                             start=False,
                                             stop=(ev == n_ev - 1),
                                             skip_group_check=True)
                            ev += 1
                    # small bias keeps pad-row denominators finite
                    nc.scalar.activation(aggAll[:, j, :], ps[:], ACTF.Copy,
                                         bias=1e-4)


            # ---------------- phase C (batched) ----------------
            # ---------------- phase C (batched, pipelined) ----------------
            with tc.tile_pool(name="pc_sb", bufs=3) as pc_sb, \
                 tc.tile_pool(name="pc_a16", bufs=3) as pc_a16, \
                 tc.tile_pool(name="pc_ps", bufs=3, space="PSUM") as pc_ps, \
                 tc.tile_pool(name="pc_ph", bufs=2, space="PSUM") as pc_ph:
                KC = 4
                ngr = (NBLK + KC - 1) // KC
                a16s = [None] * ngr

                def emit_rec(gi):
                    k = gi * KC
                    kk = min(KC, NBLK - k)
                    rec = pc_sb.tile([128, kk, H], F32, tag="rec")
                    nc.vector.reciprocal(rec[:], aggAll[:, k:k + kk, 128:130])
                    agg16 = pc_sb.tile([128, kk, H, D], FP16, tag="agg16")
                    nc.vector.tensor_tensor(
                        agg16[:],
                        aggAll[:, k:k + kk, 0:128]
                        .rearrange("p j (h d) -> p j h d", h=H),
                        rec[:].rearrange("p j (h o) -> p j h o", o=1)
                        .to_broadcast((128, kk, H, D)),
                        op=AT.mult)
                    a16s[gi] = agg16

                emit_rec(0)
                emit_rec(1)
                for gi in range(ngr):
                    if gi + 2 < ngr:
                        emit_rec(gi + 2)
                    k = gi * KC
                    kk = min(KC, NBLK - k)
                    agg16 = a16s[gi]
                    tp = pc_ps.tile([128, kk, 128], FP16, tag="tp")
                    for i in range(kk):
                        nc.tensor.transpose(
                            tp[:, i, :],
                            agg16[:, i, :, :].rearrange("p h d -> p (h d)"),
                            ident[:])
                    aggT = pc_a16.tile([128, kk, 128], FP16, tag="aggT")
                    if gi % 2 == 0:
                        nc.vector.tensor_copy(aggT[:], tp[:])
                    else:
                        nc.scalar.copy(aggT[:], tp[:])
                    o1ps = pc_ps.tile([128, kk, 128], F32, tag="o1ps")
                    for i in range(kk):
                        nc.tensor.matmul(o1ps[:, i, :], outW[:], aggT[:, i, :],
                                         start=True, stop=True)
                    o1 = pc_a16.tile([128, kk, 128], FP16, tag="o1")
                    nc.scalar.activation(o1[:], o1ps[:], ACTF.Relu,
                                         bias=outb[:, 0:1])
                    php = pc_ph.tile([128, 16], F32, tag="php")
                    for i in range(kk):
                        nc.tensor.matmul(php[:], o1[:, i, :], phWx[:, i, :],
                                         start=(i == 0), stop=(i == kk - 1),
                                         skip_group_check=True)
                    nc.vector.tensor_copy(
                        phall[:, k:k + kk, :],
                        php[:, 0:kk * P_OUT]
                        .rearrange("p (j f) -> p j f", f=P_OUT))

            with tc.tile_pool(name="pc_fin", bufs=2) as pc_sb:
                z = pc_sb.tile([128, NBLK, P_OUT], F32, tag="z")
                nc.vector.tensor_tensor(
                    z[:], phall[:],
                    phb[:].rearrange("p (o f) -> p o f", o=1)
                    .to_broadcast((128, NBLK, P_OUT)), op=AT.add)
                ez = pc_sb.tile([128, NBLK, P_OUT], F32, tag="ez")
                nc.scalar.activation(ez[:], z[:], ACTF.Exp)
                sm = pc_sb.tile([128, NBLK], F32, tag="sm")
                nc.vector.tensor_reduce(sm[:], ez[:],
                                        axis=mybir.AxisListType.X, op=AT.add)
                rc2 = pc_sb.tile([128, NBLK], F32, tag="rc2")
                nc.vector.reciprocal(rc2[:], sm[:])
                ot = pc_sb.tile([128, NBLK, P_OUT], F32, tag="ot")
                nc.vector.tensor_tensor(
                    ot[:], ez[:],
                    rc2[:].rearrange("p (j o) -> p j o", o=1)
                    .to_broadcast((128, NBLK, P_OUT)), op=AT.mult)
                nfull = NBLK - 1 if last_rows < 128 else NBLK
                nc.sync.dma_start(
                    out_d[0:nfull * 128, :].rearrange("(j p) f -> p j f", p=128),
                    ot[:, 0:nfull, :])
                if last_rows < 128:
                    nc.sync.dma_start(out_d[nfull * 128:NPC, :],
                                      ot[0:last_rows, nfull, :])

    nc.compile()
    return nc


_CACHE = {}


def kernel(**inputs) -> np.ndarray:
    meta, in_maps = prep(**inputs)
    key = "nc"
    if key not in _CACHE:
        _CACHE[key] = build(meta)
    nc = _CACHE[key]
    res = run_bass_kernel_spmd(nc, in_maps, core_ids=list(range(N_CORES)))
    out = np.concatenate([res.results[c]["out"] for c in range(N_CORES)],
                         axis=0)
    return out.astype(np.float32)


# revision 58
# speedup vs baseline: 1.0913x; 1.0913x over previous
"""GATv2 actor layer (nn_GATv2Actor) on 8 TRN2 NeuronCores via Bass/Tile.

Self-contained: kernel(**inputs) takes the full (unsharded) inputs of
reference.setup_inputs() and returns the full [50000, 4] float32 output.

Distribution strategy (edge-parallel by destination-node range):
  - node n is owned by core n // 6250; each core handles all edges whose
    destination lies in its range (plus its self-loops), so the segment
    softmax and the scatter-add are fully core-local and the final
    output rows are disjoint (host just concatenates - no collective).

Per-core program (v3 - no node-table phase; per-edge compute on PE):
  - hgT [feat, edge] fp16 arrives directly via TRANSPOSED dma_gather of
    raw h rows (128 elems each) - no PE transpose, no PSUM evacuation.
  - one-hot edge<->dst matrices (oh for the scatter, ohT for the a_dst
    select) are pure index data: host-built, streamed as fp8 {0,1}.
  - per tile: stT[hd,e] = W_src^T@hgT + adst_j@ohT (2 matmuls, f32 PSUM)
    -> relu -> lr fp16 (engine round-robin Pool/Act/DVE); lgp[e,2] =
    lr^T@(0.8*attn); lg = lgp + csum (host linear leaky_relu term);
    exx[e,2,64] = Exp(lg) broadcast (Act); v[e,128] = hgT^T@Wv (PE);
    wt = v * exx (TT from PSUM, Pool/DVE round-robin); scatter
    ps[d,0:130] += oh^T @ [wt | ex] per dst-block (PE).
  - phase C: batched softmax denominators, out MLP, phase softmax
    (unchanged from v2).

SPMD uniformity: one program runs on all 8 cores; per-(block,stream)
tile counts are padded to the max over cores. int16 gather indices
limit tables to 32767 rows, so edges are split into two streams by
src < 32768 gathering from two base offsets of the h16 table.
"""
import sys

import numpy as np

sys.path.insert(0, "/opt/trn_rl_repo")

import ml_dtypes  # noqa: E402

import concourse.bass as bass  # noqa: E402
import concourse.tile as tile  # noqa: E402
from concourse import bacc, mybir  # noqa: E402
from concourse.bass_utils import run_bass_kernel_spmd  # noqa: E402

FP16 = mybir.dt.float16
F32 = mybir.dt.float32
FP8 = mybir.dt.float8e4
I16 = mybir.dt.int16
AT = mybir.AluOpType
ACTF = mybir.ActivationFunctionType

F = 128      # feature dim
H = 2        # heads
D = 64       # head dim
P_OUT = 4    # phases
N_CORES = 8


def _to_fp8_01(mask):
    """bool -> float8_e4m3 {0.0, 1.0} without a slow dtype conversion."""
    return (mask.astype(np.uint8) * 0x38).view(ml_dtypes.float8_e4m3)


def prep(h_int, edge_index, pair_W, pair_b, attn_w, value_W, out_W, out_b,
         phase_W, phase_b, n_cores=N_CORES, G=16, split=32768):
    """Host-side index preprocessing + input packing. Returns (meta, in_maps)."""
    h = np.asarray(h_int, np.float32)
    ei = np.asarray(edge_index)
    pair_W = np.asarray(pair_W, np.float32)
    pair_b = np.asarray(pair_b, np.float32)
    attn_w = np.asarray(attn_w, np.float32)
    value_W = np.asarray(value_W, np.float32)
    out_W = np.asarray(out_W, np.float32)
    out_b = np.asarray(out_b, np.float32)
    phase_W = np.asarray(phase_W, np.float32)
    phase_b = np.asarray(phase_b, np.float32)
    N = h.shape[0]
    assert N % n_cores == 0
    NPC = N // n_cores
    NBLK = (NPC + 127) // 128
    NPAD = ((N + 127) // 128) * 128
    assert NPAD - split < 32768 and split < 32768 + 1

    # self-loops are handled per dst-block in the aggregation (v_loc matmul
    # + host exp_self scale), not as edges
    src = ei[0].astype(np.int64)
    dst = ei[1].astype(np.int64)
    core = dst // NPC

    # per-node linear logit terms: 0.2 * attn_h . a_src / 0.2 * attn_h . (a_dst+b)
    a_src_n = np.einsum('nf,hfd->nhd', h, pair_W[:, :F, :])
    a_dst_n = np.einsum('nf,hfd->nhd', h, pair_W[:, F:, :]) + pair_b[None]
    csrc_n = 0.2 * np.einsum('nhd,hd->nh', a_src_n, attn_w)     # [N, H]
    cdst_n = 0.2 * np.einsum('nhd,hd->nh', a_dst_n, attn_w)     # [N, H]
    csum_e = (csrc_n[src] + cdst_n[dst]).astype(np.float16)     # [E, H]

    # self-loop attention weight per node: exp(attn . leaky_relu(a_src+a_dst))
    st_self = a_src_n + a_dst_n                                  # [N, H, D]
    hid_self = np.where(st_self > 0, st_self, 0.2 * st_self)
    est_n = np.exp(np.einsum('nhd,hd->nh', hid_self, attn_w))    # [N, H]

    percore = []
    counts = np.zeros((n_cores, 2, NBLK), np.int64)
    for c in range(n_cores):
        m = core == c
        es = src[m]
        ed = dst[m] - c * NPC
        cs = csum_e[m]
        o = np.lexsort((es, ed))
        es, ed, cs = es[o], ed[o], cs[o]
        lo = es < split
        percore.append((es, ed, cs, lo))
        for si in range(2):
            msk = lo if si == 0 else ~lo
            counts[c, si] = np.bincount(ed[msk] // 128, minlength=NBLK)
    T = np.ceil(counts.max(axis=0) / 128.0).astype(np.int64)  # [2, NBLK]
    tiles = T.sum(axis=1)
    L = tiles * 128
    base_tile = np.zeros((2, NBLK + 1), np.int64)
    base_tile[:, 1:] = np.cumsum(T, axis=1)

    f16 = np.float16
    # weights: columns are (head, d) flattened, matching agg.reshape(n,-1)
    W_src = np.concatenate([pair_W[0, :F], pair_W[1, :F]], axis=1).astype(f16)
    Wv = np.concatenate([value_W[0], value_W[1]], axis=1).astype(f16)
    ident16 = np.eye(128, dtype=f16)
    S16 = np.zeros((128, H), f16)
    for hh in range(H):
        S16[hh * D:(hh + 1) * D, hh] = (0.8 * attn_w[hh]).astype(f16)
    out_Wt = np.asarray(out_W, f16)
    out_b_c = out_b.reshape(128, 1).copy()
    phase_Wt = np.asarray(phase_W, f16)
    phase_b_bc = np.broadcast_to(phase_b, (128, P_OUT)).copy()

    hp = np.zeros((NPAD, F), np.float32)
    hp[:N] = h
    h16 = hp.astype(f16)
    h16_lo = np.ascontiguousarray(h16[:split])
    h16_hi = np.ascontiguousarray(h16[split:])

    # host adst table (a_dst + bias), per-core local blocks [128, NBLK, 128]
    adp = np.zeros((NPAD, H * D), np.float32)
    adp[:N] = a_dst_n.reshape(N, H * D)

    # zero-padded 130-col copies of Wv per head + 2x2 identity for the
    # self-loop matmuls (each covers the full agg range so PSUM group
    # init is a single full-width start=True write)
    Wv0x = np.zeros((128, 130), f16)
    Wv0x[:, 0:D] = Wv[:, 0:D]
    Wv1x = np.zeros((128, 130), f16)
    Wv1x[:, D:2 * D] = Wv[:, D:2 * D]
    I2x = np.zeros((2, 130), f16)
    I2x[0, 128] = 1.0
    I2x[1, 129] = 1.0

    # phase_W copies padded to a 16-col stripe per block-within-group so
    # the four phase matmuls of a group accumulate into one PSUM region
    # with a single full-width start=True (sim/hw-safe group init)
    phWx = np.zeros((4, 128, 16), f16)
    for i in range(4):
        phWx[i, :, i * P_OUT:(i + 1) * P_OUT] = phase_W.astype(f16)

    shared = dict(h16_lo=h16_lo, h16_hi=h16_hi, W_src=W_src, Wv=Wv,
                  Wv0x=Wv0x, Wv1x=Wv1x, I2x=I2x,
                  phWx=phWx.reshape(4 * 128, 16),
                  ident16=ident16, S16=S16, out_Wt=out_Wt,
                  out_b=out_b_c, phase_b_bc=phase_b_bc)

    dcols = np.arange(128, dtype=np.int64)
    in_maps = []
    for c in range(n_cores):
        es, ed, cs, lo = percore[c]
        m = {}
        for si in range(2):
            msk = lo if si == 0 else ~lo
            es_s, ed_s, cs_s = es[msk], ed[msk], cs[msk]
            gidx = np.zeros(L[si], np.int16)
            drel = np.full(L[si], -1.0, np.float32)
            csum = np.zeros((L[si], H), np.float16)
            starts = np.searchsorted(ed_s, np.arange(NBLK + 1) * 128)
            for j in range(NBLK):
                seg = slice(starts[j], starts[j + 1])
                n = starts[j + 1] - starts[j]
                b0 = base_tile[si, j] * 128
                gidx[b0:b0 + n] = (es_s[seg] - (split if si else 0)).astype(np.int16)
                drel[b0:b0 + n] = (ed_s[seg] - j * 128).astype(np.float32)
                csum[b0:b0 + n] = cs_s[seg]
            m[f"gw{si}"] = np.tile(gidx.reshape(-1, 16).T, (8, 1)).copy()
            m[f"csum{si}"] = np.ascontiguousarray(
                csum.reshape(-1, 128, H).transpose(1, 0, 2).reshape(128, -1))
            d2 = drel.reshape(-1, 128)                      # [T, e]
            oh = _to_fp8_01(d2[:, :, None] == dcols[None, None, :])  # [T,e,d]
            m[f"oh8_{si}"] = np.ascontiguousarray(
                oh.transpose(1, 0, 2)).reshape(128, -1)
            m[f"ohT8_{si}"] = np.ascontiguousarray(
                oh.transpose(2, 0, 1)).reshape(128, -1)
        adl = adp[c * NPC:c * NPC + NBLK * 128]
        m["adst"] = np.ascontiguousarray(
            adl.reshape(NBLK, 128, 128).transpose(1, 0, 2)
            .reshape(128, -1).astype(f16))
        # self-loop data: local h pre-scaled by exp_self per head
        # (feature-major), plus exp_self itself for the denominators
        hl = np.zeros((NBLK * 128, F), np.float32)
        hl[:NPC] = h[c * NPC:(c + 1) * NPC]
        estl = np.zeros((NBLK * 128, H), np.float32)
        estl[:NPC] = est_n[c * NPC:(c + 1) * NPC]
        for hh in range(H):
            m[f"hsc{hh}"] = np.ascontiguousarray(
                (hl * estl[:, hh:hh + 1]).T.astype(f16))
        m["estT2"] = np.ascontiguousarray(estl.T.astype(f16))
        m.update(shared)
        in_maps.append(m)

    meta = dict(N=N, NPC=NPC, NBLK=NBLK, NPAD=NPAD, split=split, G=G,
                T=T, tiles=tiles, L=L, base_tile=base_tile,
                n_cores=n_cores)
    return meta, in_maps


def build(meta):
    NPC, NBLK, NPAD = meta["NPC"], meta["NBLK"], meta["NPAD"]
    split, G = meta["split"], meta["G"]
    T, tiles, L = meta["T"], meta["tiles"], meta["L"]
    base_tile = meta["base_tile"]
    last_rows = NPC - (NBLK - 1) * 128

    # tile index -> destination block, per stream (compile-time constants,
    # shared across cores by construction of T)
    tile2blk = [[], []]
    for s in range(2):
        for j in range(NBLK):
            tile2blk[s].extend([j] * int(T[s][j]))

    nc = bacc.Bacc(None, target_bir_lowering=False, debug=False,
                   dynamic_dma_scratch_size=98304)

    hlo_d = nc.dram_tensor("h16_lo", [split, F], FP16, kind="ExternalInput")
    hhi_d = nc.dram_tensor("h16_hi", [NPAD - split, F], FP16,
                           kind="ExternalInput")
    gw_d = [nc.dram_tensor(f"gw{s}", [128, int(L[s]) // 16], I16,
                           kind="ExternalInput") for s in range(2)]
    oh8_d = [nc.dram_tensor(f"oh8_{s}", [128, int(tiles[s]) * 128], FP8,
                            kind="ExternalInput") for s in range(2)]
    ohT8_d = [nc.dram_tensor(f"ohT8_{s}", [128, int(tiles[s]) * 128], FP8,
                             kind="ExternalInput") for s in range(2)]
    csum_d = [nc.dram_tensor(f"csum{s}", [128, int(tiles[s]) * H], FP16,
                             kind="ExternalInput") for s in range(2)]
    adst_d = nc.dram_tensor("adst", [128, NBLK * 128], FP16,
                            kind="ExternalInput")
    hsc_d = [nc.dram_tensor(f"hsc{hh}", [128, NBLK * 128], FP16,
                            kind="ExternalInput") for hh in range(H)]
    estT2_d = nc.dram_tensor("estT2", [2, NBLK * 128], FP16,
                             kind="ExternalInput")
    Wv0x_d = nc.dram_tensor("Wv0x", [128, 130], FP16, kind="ExternalInput")
    Wv1x_d = nc.dram_tensor("Wv1x", [128, 130], FP16, kind="ExternalInput")
    I2x_d = nc.dram_tensor("I2x", [2, 130], FP16, kind="ExternalInput")
    Wsrc_d = nc.dram_tensor("W_src", [128, 128], FP16, kind="ExternalInput")
    Wv_d = nc.dram_tensor("Wv", [128, 128], FP16, kind="ExternalInput")
    S_d = nc.dram_tensor("S16", [128, H], FP16, kind="ExternalInput")
    ident_d = nc.dram_tensor("ident16", [128, 128], FP16, kind="ExternalInput")
    outW_d = nc.dram_tensor("out_Wt", [128, 128], FP16, kind="ExternalInput")
    outb_d = nc.dram_tensor("out_b", [128, 1], F32, kind="ExternalInput")
    phWx_d = nc.dram_tensor("phWx", [4 * 128, 16], FP16, kind="ExternalInput")
    phb_d = nc.dram_tensor("phase_b_bc", [128, P_OUT], F32, kind="ExternalInput")

    out_d = nc.dram_tensor("out", [NPC, P_OUT], F32, kind="ExternalOutput")

    with tile.TileContext(nc) as tc:
        with tc.tile_pool(name="consts", bufs=1) as pc:
            def cload(name, dram, shape, dtype):
                t = pc.tile(shape, dtype, tag=name)
                nc.sync.dma_start(t[:], dram[:])
                return t
            # gw first, split so the first chunks' indices land in ~100ns
            # and the opening gathers aren't gated on the full-table load
            GW_HEAD = 4 * G * 8   # first 4 chunks per stream
            gw = []
            for s in range(2):
                t = pc.tile([128, int(L[s]) // 16], I16, tag=f"gw{s}",
                            name=f"gw{s}")
                hd = min(GW_HEAD, int(L[s]) // 16)
                nc.sync.dma_start(t[:, 0:hd], gw_d[s][:, 0:hd])
                if hd < int(L[s]) // 16:
                    nc.sync.dma_start(t[:, hd:], gw_d[s][:, hd:])
                gw.append(t)
            Wsrc = cload("Wsrc", Wsrc_d, [128, 128], FP16)
            Wv = cload("Wv", Wv_d, [128, 128], FP16)
            adst = pc.tile([128, NBLK, 128], FP16, tag="adst")
            nc.sync.dma_start(
                adst[:], adst_d[:].rearrange("p (j n) -> p j n", j=NBLK))
            S = cload("S", S_d, [128, H], FP16)
            csum = [pc.tile([128, int(tiles[s]), H], FP16, tag=f"csum{s}",
                            name=f"csum{s}") for s in range(2)]
            for s in range(2):
                nc.sync.dma_start(
                    csum[s][:],
                    csum_d[s][:].rearrange("p (t h) -> p t h", h=H))
            Wv0x = cload("Wv0x", Wv0x_d, [128, 130], FP16)
            Wv1x = cload("Wv1x", Wv1x_d, [128, 130], FP16)
            I2x = cload("I2x", I2x_d, [2, 130], FP16)
            ident = cload("ident", ident_d, [128, 128], FP16)
            ident32 = pc.tile([128, 128], F32, tag="ident32")
            nc.vector.tensor_copy(ident32[:], ident[:])
            outW = cload("outW", outW_d, [128, 128], FP16)
            outb = cload("outb", outb_d, [128, 1], F32)
            phWx = pc.tile([128, 4, 16], FP16, tag="phWx")
            nc.sync.dma_start(phWx[:],
                              phWx_d[:].rearrange("(i p) f -> p i f", i=4))
            phb = cload("phb", phb_d, [128, P_OUT], F32)
            aggAll = pc.tile([128, NBLK, 130], FP16, tag="aggAll")
            phall = pc.tile([128, NBLK, P_OUT], F32, tag="phall")

            h_base = [hlo_d[:], hhi_d[:]]

            with tc.tile_pool(name="pg_hgT", bufs=6) as pg_hgT, \
                 tc.tile_pool(name="p_oh", bufs=5) as p_oh, \
                 tc.tile_pool(name="p_ohT", bufs=4) as p_ohT, \
                 tc.tile_pool(name="p_lr", bufs=3) as p_lr, \
                 tc.tile_pool(name="p_lg", bufs=4) as p_lg, \
                 tc.tile_pool(name="p_exx", bufs=3) as p_exx, \
                 tc.tile_pool(name="p_wt", bufs=3) as p_wt, \
                 tc.tile_pool(name="p_self", bufs=3) as p_self, \
                 tc.tile_pool(name="pc_work", bufs=2) as pc_work, \
                 tc.tile_pool(name="ps_stT", bufs=3, space="PSUM") as ps_stT, \
                 tc.tile_pool(name="ps_v", bufs=2, space="PSUM") as ps_v, \
                 tc.tile_pool(name="ps_lg", bufs=1, space="PSUM") as ps_lg, \
                 tc.tile_pool(name="ps_agg", bufs=2, space="PSUM") as ps_agg:

                # relu / wt engine round-robin (tunable balance knobs)
                def relu_pool(out, in_):
                    nc.gpsimd.tensor_scalar(out, in_, 0.0, None, op0=AT.max)

                def relu_act(out, in_):
                    nc.scalar.activation(out, in_, ACTF.Relu)

                def relu_dve(out, in_):
                    nc.vector.tensor_scalar(out, in_, 0.0, None, op0=AT.max)

                # Pool cannot touch PSUM (walrus birverifier) - PSUM
                # evacuations go DVE/Act only.
                relu_rr = [relu_act, relu_act, relu_act, relu_dve]

                def wt_dve(out, a, b):
                    nc.vector.tensor_tensor(out, a, b, op=AT.mult)

                def wt_pool(out, a, b):
                    nc.gpsimd.tensor_tensor(out, a, b, op=AT.mult)

                wt_rr = [wt_dve, wt_dve, wt_dve, wt_dve]

                rr_ctr = [0, 0, 0, 0]
                state = [dict(), dict()]

                def stage1(s, ci):
                    if ci in state[s]:
                        return
                    t0 = ci * G
                    g = min(G, int(tiles[s]) - t0)
                    hgT = pg_hgT.tile([128, g, 128], FP16, tag="hgT",
                                      name="hgT")
                    GSUB = 4  # transpose-mode gathers fail above 512 idxs
                    for k in range(0, g, GSUB):
                        gs = min(GSUB, g - k)
                        ne = gs * 128
                        nc.gpsimd.dma_gather(
                            hgT[:, k:k + gs, :]
                            .rearrange("p t e -> p (t e)")
                            .rearrange("p (o n) -> p o n", o=1),
                            h_base[s],
                            gw[s][:, (t0 + k) * 8:(t0 + k + gs) * 8],
                            ne, ne, 128, transpose=True)
                    oh8 = p_oh.tile([128, g, 128], FP8, tag="oh8",
                                    name="oh8")
                    nc.sync.dma_start(
                        oh8[:], oh8_d[s][:, t0 * 128:(t0 + g) * 128]
                        .rearrange("p (t e) -> p t e", t=g))
                    ohT8 = p_ohT.tile([128, g, 128], FP8, tag="ohT8",
                                      name="ohT8")
                    nc.sync.dma_start(
                        ohT8[:], ohT8_d[s][:, t0 * 128:(t0 + g) * 128]
                        .rearrange("p (t e) -> p t e", t=g))
                    state[s][ci] = dict(stg=1, t0=t0, g=g, hgT=hgT,
                                        oh=oh8, ohT=ohT8)

                def stage2(s, ci):
                    stage1(s, ci)
                    st = state[s][ci]
                    if st["stg"] >= 2:
                        return
                    t0, g, hgT, ohT8 = st["t0"], st["g"], st["hgT"], st["ohT"]
                    lr = p_lr.tile([128, g, 128], FP16, tag="lr", name="lr")
                    lgp = ps_lg.tile([128, g, H], F32, tag="lgp", name="lgp")
                    nk = (g + 3) // 4

                    def emit_lgp(kidx):
                        k = kidx * 4
                        kk = min(4, g - k)
                        for i in range(kk):
                            t = k + i
                            nc.tensor.matmul(lgp[:, t, :], lr[:, t, :], S[:],
                                             start=True, stop=True)

                    # lgp trails the relu by one 4-tile batch so the PE queue
                    # never parks on an in-flight relu
                    for kidx in range(nk):
                        k = kidx * 4
                        kk = min(4, g - k)
                        stT = ps_stT.tile([128, kk, 128], F32, tag="stT",
                                          name="stT")
                        # one W_src matmul covers the whole 4-tile batch
                        nc.tensor.matmul(
                            stT[:], Wsrc[:],
                            hgT[:, k:k + kk, :],
                            start=True, stop=False, skip_group_check=True)
                        for i in range(kk):
                            t = k + i
                            j = tile2blk[s][t0 + t]
                            nc.tensor.matmul(stT[:, i, :], adst[:, j, :],
                                             ohT8[:, t, :],
                                             start=False, stop=(i == kk - 1),
                                             skip_group_check=True)
                        relu_rr[rr_ctr[0] % len(relu_rr)](lr[:, k:k + kk, :],
                                                          stT[:])
                        rr_ctr[0] += 1
                        if kidx >= 1:
                            emit_lgp(kidx - 1)
                    emit_lgp(nk - 1)
                    # evacuate lgp immediately (adds the host linear
                    # leaky_relu term) so ps_lg recycles with one buf
                    lg = p_lg.tile([128, g, H], FP16, tag="lg", name="lg")
                    nc.vector.tensor_tensor(lg[:], lgp[:],
                                            csum[s][:, t0:t0 + g, :],
                                            op=AT.add)
                    st["lg"] = lg
                    st["stg"] = 2

                def stage3a(s, ci):
                    stage2(s, ci)
                    st = state[s][ci]
                    if st["stg"] >= 3:
                        return
                    g, lg = st["g"], st["lg"]
                    ex32 = p_lg.tile([128, g, H], FP16, tag="ex32",
                                     name="ex32")
                    nc.scalar.activation(ex32[:], lg[:], ACTF.Exp)
                    exx = p_exx.tile([128, g, H, D], FP16, tag="exx",
                                     name="exx")
                    bc = (ex32[:].rearrange("p g (h o) -> p g h o", o=1)
                          .to_broadcast((128, g, H, D)))
                    if rr_ctr[2] % 5 < 3:
                        nc.gpsimd.tensor_copy(exx[:], bc)
                    else:
                        nc.scalar.copy(exx[:], bc)
                    rr_ctr[2] += 1
                    st["exx"], st["ex32"] = exx, ex32
                    st["stg"] = 3

                def stage3b(s, ci):
                    stage3a(s, ci)
                    st = state[s][ci]
                    if st["stg"] >= 4:
                        return st["oh"], st["wt"]
                    g, hgT, exx = st["g"], st["hgT"], st["exx"]
                    wt = p_wt.tile([128, g, 130], FP16, tag="wt", name="wt")
                    nk = (g + 3) // 4
                    for kidx in range(nk):
                        k = kidx * 4
                        kk = min(4, g - k)
                        v_ps = ps_v.tile([128, kk, 128], F32, tag="v_ps",
                                         name="v_ps")
                        for i in range(kk):
                            nc.tensor.matmul(v_ps[:, i, :], hgT[:, k + i, :],
                                             Wv[:], start=True, stop=True)
                        wt_rr[rr_ctr[1] % len(wt_rr)](
                            wt[:, k:k + kk, 0:128], v_ps[:],
                            exx[:, k:k + kk, :, :]
                            .rearrange("p g h d -> p g (h d)"))
                        rr_ctr[1] += 1
                    nc.gpsimd.tensor_copy(wt[:, :, 128:130], st["ex32"][:])
                    st["wt"] = wt
                    st["stg"] = 4
                    return st["oh"], st["wt"]

                # self-loop per-block prefetch (pre-scaled hloc + exp_self)
                self_tiles = [None] * NBLK

                def self_fetch(j):
                    if j >= NBLK or self_tiles[j] is not None:
                        return
                    hs0 = p_self.tile([128, 128], FP16, tag="hs0", name="hs0")
                    nc.sync.dma_start(hs0[:],
                                      hsc_d[0][:, j * 128:(j + 1) * 128])
                    hs1 = p_self.tile([128, 128], FP16, tag="hs1", name="hs1")
                    nc.sync.dma_start(hs1[:],
                                      hsc_d[1][:, j * 128:(j + 1) * 128])
                    es = p_self.tile([2, 128], FP16, tag="es", name="es")
                    nc.sync.dma_start(es[:],
                                      estT2_d[:, j * 128:(j + 1) * 128])
                    self_tiles[j] = (hs0, hs1, es)

                # global tile sequence in scatter order, for chunk prefetch
                seq = [(j, s, int(base_tile[s, j]) + t)
                       for j in range(NBLK)
                       for s in range(2)
                       for t in range(int(T[s][j]))]
                L3A, L2, L1 = 12, 32, 48   # lookahead in tiles per stage
                si = 0
                self_fetch(0)
                self_fetch(1)
                for j in range(NBLK):
                    self_fetch(j + 2)
                    n_ev = int(T[0][j] + T[1][j]) + 3
                    ps = ps_agg.tile([128, 130], F32, tag="ps")
                    hs0, hs1, es = self_tiles[j]
                    self_tiles[j] = None
                    # self-loop contribution: agg[d] += exp_self_h[d] *
                    # (h[d] @ Wv_h), denominators += exp_self, each matmul
                    # spanning the full 130 cols via zero-padded weights
                    nc.tensor.matmul(ps[:], hs0[:], Wv0x[:],
                                     start=True, stop=False,
                                     skip_group_check=True)
                    nc.tensor.matmul(ps[:], hs1[:], Wv1x[:],
                                     start=False, stop=False,
                                     skip_group_check=True)
                    nc.tensor.matmul(ps[:], es[:], I2x[:],
                                     start=False, stop=False,
                                     skip_group_check=True)
                    ev = 3
                    for s in range(2):
                        for t in range(int(T[s][j])):
                            gt = int(base_tile[s, j]) + t
                            if si + L3A < len(seq):
                                _, s3a, gt3a = seq[si + L3A]
                                stage3a(s3a, gt3a // G)
                            if si + L2 < len(seq):
                                _, s2, gt2 = seq[si + L2]
                                stage2(s2, gt2 // G)
                            if si + L1 < len(seq):
                                _, s3, gt3 = seq[si + L1]
                                stage1(s3, gt3 // G)
                            oh, wt = stage3b(s, gt // G)
                            si += 1
                            off = gt % G
                            nc.tensor.matmul(ps[:], oh[:, off, :],
                                             wt[:, off, 0:130],
                                             start=False,
                                             stop=(ev == n_ev - 1),
                                             skip_group_check=True)
                            ev += 1
                    # small bias keeps pad-row denominators finite
                    nc.scalar.activation(aggAll[:, j, :], ps[:], ACTF.Copy,
                                         bias=1e-4)


            # ---------------- phase C (batched) ----------------
            # ---------------- phase C (batched, pipelined) ----------------
            with tc.tile_pool(name="pc_sb", bufs=3) as pc_sb, \
                 tc.tile_pool(name="pc_a16", bufs=3) as pc_a16, \
                 tc.tile_pool(name="pc_ps", bufs=3, space="PSUM") as pc_ps, \
                 tc.tile_pool(name="pc_ph", bufs=2, space="PSUM") as pc_ph:
                KC = 4
                ngr = (NBLK + KC - 1) // KC
                a16s = [None] * ngr

                def emit_rec(gi):
                    k = gi * KC
                    kk = min(KC, NBLK - k)
                    rec = pc_sb.tile([128, kk, H], F32, tag="rec")
                    nc.vector.reciprocal(rec[:], aggAll[:, k:k + kk, 128:130])
                    agg16 = pc_sb.tile([128, kk, H, D], FP16, tag="agg16")
                    nc.vector.tensor_tensor(
                        agg16[:],
                        aggAll[:, k:k + kk, 0:128]
                        .rearrange("p j (h d) -> p j h d", h=H),
                        rec[:].rearrange("p j (h o) -> p j h o", o=1)
                        .to_broadcast((128, kk, H, D)),
                        op=AT.mult)
                    a16s[gi] = agg16

                emit_rec(0)
                emit_rec(1)
                for gi in range(ngr):
                    if gi + 2 < ngr:
                        emit_rec(gi + 2)
                    k = gi * KC
                    kk = min(KC, NBLK - k)
                    agg16 = a16s[gi]
                    tp = pc_ps.tile([128, kk, 128], FP16, tag="tp")
                    for i in range(kk):
                        nc.tensor.transpose(
                            tp[:, i, :],
                            agg16[:, i, :, :].rearrange("p h d -> p (h d)"),
                            ident[:])
                    aggT = pc_a16.tile([128, kk, 128], FP16, tag="aggT")
                    if gi % 2 == 0:
                        nc.vector.tensor_copy(aggT[:], tp[:])
                    else:
                        nc.scalar.copy(aggT[:], tp[:])
                    o1ps = pc_ps.tile([128, kk, 128], F32, tag="o1ps")
                    for i in range(kk):
                        nc.tensor.matmul(o1ps[:, i, :], outW[:], aggT[:, i, :],
                                         start=True, stop=True)
                    o1 = pc_a16.tile([128, kk, 128], FP16, tag="o1")
                    nc.scalar.activation(o1[:], o1ps[:], ACTF.Relu,
                                         bias=outb[:, 0:1])
                    php = pc_ph.tile([128, 16], F32, tag="php")
                    for i in range(kk):
                        nc.tensor.matmul(php[:], o1[:, i, :], phWx[:, i, :],
                                         start=(i == 0), stop=(i == kk - 1),
                                         skip_group_check=True)
                    nc.vector.tensor_copy(
                        phall[:, k:k + kk, :],
                        php[:, 0:kk * P_OUT]
                        .rearrange("p (j f) -> p j f", f=P_OUT))

            with tc.tile_pool(name="pc_fin", bufs=2) as pc_sb:
                z = pc_sb.tile([128, NBLK, P_OUT], F32, tag="z")
                nc.vector.tensor_tensor(
                    z[:], phall[:],
                    phb[:].rearrange("p (o f) -> p o f", o=1)
                    .to_broadcast((128, NBLK, P_OUT)), op=AT.add)
                ez = pc_sb.tile([128, NBLK, P_OUT], F32, tag="ez")
                nc.scalar.activation(ez[:], z[:], ACTF.Exp)
                sm = pc_sb.tile([128, NBLK], F32, tag="sm")
                nc.vector.tensor_reduce(sm[:], ez[:],
                                        axis=mybir.AxisListType.X, op=AT.add)
                rc2 = pc_sb.tile([128, NBLK], F32, tag="rc2")
                nc.vector.reciprocal(rc2[:], sm[:])
                ot = pc_sb.tile([128, NBLK, P_OUT], F32, tag="ot")
                nc.vector.tensor_tensor(
                    ot[:], ez[:],
                    rc2[:].rearrange("p (j o) -> p j o", o=1)
                    .to_broadcast((128, NBLK, P_OUT)), op=AT.mult)
                nfull = NBLK - 1 if last_rows < 128 else NBLK
                nc.sync.dma_start(
                    out_d[0:nfull * 128, :].rearrange("(j p) f -> p j f", p=128),
                    ot[:, 0:nfull, :])
                if last_rows < 128:
                    nc.sync.dma_start(out_d[nfull * 128:NPC, :],
                                      ot[0:last_rows, nfull, :])

    nc.compile()
    return nc


_CACHE = {}


def kernel(**inputs) -> np.ndarray:
    meta, in_maps = prep(**inputs)
    key = "nc"
    if key not in _CACHE:
        _CACHE[key] = build(meta)
    nc = _CACHE[key]
    res = run_bass_kernel_spmd(nc, in_maps, core_ids=list(range(N_CORES)))
    out = np.concatenate([res.results[c]["out"] for c in range(N_CORES)],
                         axis=0)
    return out.astype(np.float32)
